# revision 22
# baseline (speedup 1.0000x reference)
"""Trainium2 Bass kernel for nn_AngleFreqEnhance.

Reference computation (per batch element b):
    x_proj = w_in @ x_b                    # (16, 256) @ (256, 16384)
    Z      = fftshift(fft2(x_proj, ortho))
    enh    = (|Z|+eps) * gain * Z/|Z|      # == gain * Z up to ~1e-8 absolute
    x_enh  = ifft2(ifftshift(enh), ortho).real
    out_b  = x_b + w_out @ x_enh           # (256, 16) @ (16, 16384)

Device formulation:
  * eps term dropped (absolute error <= gain * 1e-8, far below the 2e-2 gate).
  * fftshift/ifftshift folded into the gain map (host-side ifftshift of the
    constant angle/radius selection maps).
  * 2-D FFT/IFFT done as matmuls with the symmetric 128x128 ortho DFT matrix
    F = Fr + i*Fi (bf16 operands, fp32 PSUM accumulate).  Each matmul stage
    contracts the partition axis and transposes the kept axis, so four stages
    (fwd row, fwd col, inv row, inv col) chain with no explicit transposes.
  * gain = bin_weights-flat @ Wsel, where Wsel[(a,r), pix] is the
    host-precomputed constant angle-weight * radius-indicator map
    (shape-derived constants only; bin_weights stays a device input).
  * All matmul operands are bf16 (full-rate TensorE + fast weight load); the
    residual x stream, gain map, PSUM accumulation, and output stay fp32, so
    bf16 rounding only touches the small enhancement term.

Sharding: pure data parallel - batch element b on core b (B=8, 8 cores).
No collectives. Host gathers per-core outputs.
"""

import math

import numpy as np

B, C_IN, C_MID, H, W = 8, 256, 16, 128, 128
HW = H * W
N_ANGLES = 8
N_RADII = 9
AR = N_ANGLES * N_RADII  # 72
CH = 2048                # pixel chunk for the streaming passes
NCH = HW // CH           # 8
NS = CH // 512           # 4 (PSUM-bank sized sub-chunks)

_CACHE = {}


def _build_masks():
    """Bit-faithful replica of the reference _build_masks on CPU jax.

    The reference's `(arctan2 + pi) % pi` lowers to an IEEE-remainder on
    XLA:CPU (range (-pi/2, pi/2]), which zeroes the angle weights over half
    the plane.  Running the identical jnp ops on the identical backend is the
    only safe way to reproduce the oracle.
    """
    import jax
    import jax.numpy as jnp

    EPS = 1e-8
    RADIUS_WIDTH = 8
    OVERLAP_RATIO = 1.5
    with jax.default_device(jax.devices("cpu")[0]):
        cy, cx = H // 2, W // 2
        y = jnp.arange(H, dtype=jnp.float32)[:, None] - cy
        x = jnp.arange(W, dtype=jnp.float32)[None, :] - cx
        r = jnp.sqrt(y * y + x * x)
        theta = (jnp.arctan2(y, x) + math.pi) % math.pi
        n_radii = int(max(cy, cx) // RADIUS_WIDTH) + 1
        radius_idx = jnp.clip(
            jnp.floor(r / RADIUS_WIDTH).astype(jnp.int32), 0, n_radii - 1
        )
        delta = math.pi / N_ANGLES
        half_width = OVERLAP_RATIO * delta / 2.0
        centers = (jnp.arange(N_ANGLES, dtype=jnp.float32) * delta + delta / 2.0)[
            :, None, None
        ]
        dist = jnp.abs(theta[None] - centers)
        w = jnp.clip(1.0 - dist / half_width, 0.0) * (dist < half_width)
        angle_weights = w / (w.sum(axis=0, keepdims=True) + EPS)
        return np.asarray(radius_idx), np.asarray(angle_weights)


def _host_constants():
    if "consts" in _CACHE:
        return _CACHE["consts"]
    import ml_dtypes

    bf = ml_dtypes.bfloat16
    radius_idx, aw = _build_masks()
    maps = aw[:, None] * (
        radius_idx[None, None] == np.arange(N_RADII)[None, :, None, None]
    ).astype(np.float32)
    wsel = (
        np.fft.ifftshift(maps, axes=(-2, -1))
        .reshape(AR, HW)
        .astype(np.float32)
    )
    k = np.arange(H)
    th = 2.0 * np.pi * np.outer(k, k) / H
    fr = (np.cos(th) / math.sqrt(H)).astype(np.float32)
    fi = (-np.sin(th) / math.sqrt(H)).astype(np.float32)
    consts = (
        np.ascontiguousarray(wsel.astype(bf)),
        np.ascontiguousarray(fr.astype(bf)),
        np.ascontiguousarray(fi.astype(bf)),
        np.ascontiguousarray((-fi).astype(bf)),
    )
    _CACHE["consts"] = consts
    return consts


def _build_nc():
    if "nc" in _CACHE:
        return _CACHE["nc"]
    import concourse.bass as bass
    import concourse.bacc as bacc
    import concourse.tile as tile
    from concourse import mybir

    f32 = mybir.dt.float32
    bf16 = mybir.dt.bfloat16
    PSUM = bass.MemorySpace.PSUM

    nc = bacc.Bacc(
        None,
        target_bir_lowering=False,
        debug=False,
        enable_asserts=False,
        num_devices=B,
    )

    x_d = nc.declare_dram_parameter("x", [C_IN, HW], f32, isOutput=False)
    w_inT_d = nc.declare_dram_parameter("w_inT", [C_IN, C_MID], bf16, isOutput=False)
    w_outT_d = nc.declare_dram_parameter("w_outT", [C_MID, C_IN], bf16, isOutput=False)
    binT_d = nc.declare_dram_parameter("binT", [AR, C_MID], bf16, isOutput=False)
    wsel_d = nc.declare_dram_parameter("wsel", [AR, HW], bf16, isOutput=False)
    fr_d = nc.declare_dram_parameter("fr", [H, H], bf16, isOutput=False)
    fi_d = nc.declare_dram_parameter("fi", [H, H], bf16, isOutput=False)
    fineg_d = nc.declare_dram_parameter("fineg", [H, H], bf16, isOutput=False)
    out_d = nc.declare_dram_parameter("out", [C_IN, HW], f32, isOutput=True)

    xproj_d = nc.dram_tensor("xproj_d", [C_MID, HW], bf16)
    xmid_d = nc.dram_tensor("xmid_d", [C_MID, HW], bf16)
    gain_d = nc.dram_tensor("gain_d", [C_MID, HW], bf16)

    xproj_r = xproj_d.ap().rearrange("m (h w) -> m h w", h=H)
    xmid_r = xmid_d.ap().rearrange("m (h w) -> m h w", h=H)
    gain_r = gain_d.ap().rearrange("m (h w) -> m h w", h=H)

    with tile.TileContext(nc) as tc:
        with (
            tc.tile_pool(name="const", bufs=1) as cpool,
            tc.tile_pool(name="gw", bufs=2) as gwp,
            tc.tile_pool(name="p1x", bufs=3) as p1x,
            tc.tile_pool(name="p1s", bufs=2) as p1s,
            tc.tile_pool(name="fftb", bufs=1) as fftb,
            tc.tile_pool(name="p3m", bufs=2) as p3m,
            tc.tile_pool(name="p3x", bufs=6) as p3x,
            tc.tile_pool(name="p3o", bufs=2) as p3o,
            tc.tile_pool(name="ps", bufs=8, space=PSUM) as psp,
        ):
            # ---- constants into SBUF
            w_inT_t = cpool.tile([128, 2 * C_MID], bf16)
            nc.sync.dma_start(out=w_inT_t[:, 0:C_MID], in_=w_inT_d[0:128, :])
            nc.sync.dma_start(out=w_inT_t[:, C_MID : 2 * C_MID], in_=w_inT_d[128:256, :])
            w_outT_t = cpool.tile([C_MID, C_IN], bf16)
            nc.sync.dma_start(out=w_outT_t[:], in_=w_outT_d[:])
            binT_t = cpool.tile([AR, C_MID], bf16)
            nc.sync.dma_start(out=binT_t[:], in_=binT_d[:])
            fr_t = cpool.tile([H, H], bf16)
            nc.sync.dma_start(out=fr_t[:], in_=fr_d[:])
            fi_t = cpool.tile([H, H], bf16)
            nc.sync.dma_start(out=fi_t[:], in_=fi_d[:])
            fng_t = cpool.tile([H, H], bf16)
            nc.sync.dma_start(out=fng_t[:], in_=fineg_d[:])
            # paired moving operands for batched FFT matmuls
            Fri = cpool.tile([H, 2 * H], bf16)   # [Fr | Fi]
            nc.sync.dma_start(out=Fri[:, 0:H], in_=fr_d[:])
            nc.sync.dma_start(out=Fri[:, H : 2 * H], in_=fi_d[:])
            Fnr = cpool.tile([H, 2 * H], bf16)   # [-Fi | Fr]
            nc.sync.dma_start(out=Fnr[:, 0:H], in_=fineg_d[:])
            nc.sync.dma_start(out=Fnr[:, H : 2 * H], in_=fr_d[:])
            Frn = cpool.tile([H, 2 * H], bf16)   # [Fr | -Fi]
            nc.sync.dma_start(out=Frn[:, 0:H], in_=fr_d[:])
            nc.sync.dma_start(out=Frn[:, H : 2 * H], in_=fineg_d[:])
            Fir = cpool.tile([H, 2 * H], bf16)   # [Fi | Fr]
            nc.sync.dma_start(out=Fir[:, 0:H], in_=fi_d[:])
            nc.sync.dma_start(out=Fir[:, H : 2 * H], in_=fr_d[:])

            # ---- gain table (computed up front, overlaps the x stream)
            for c in range(NCH):
                wt = gwp.tile([AR, CH], bf16)
                nc.sync.dma_start(out=wt[:], in_=wsel_d[:, c * CH : (c + 1) * CH])
                gs = p1s.tile([C_MID, CH], bf16, tag="gs")
                for s in range(NS):
                    pg = psp.tile([C_MID, 512], f32, tag="ps")
                    nc.tensor.matmul(
                        pg[:],
                        binT_t[:],
                        wt[:, s * 512 : (s + 1) * 512],
                        start=True,
                        stop=True,
                    )
                    nc.vector.tensor_copy(gs[:, s * 512 : (s + 1) * 512], pg[:])
                nc.scalar.dma_start(out=gain_d[:, c * CH : (c + 1) * CH], in_=gs[:])

            # ---- phase 1: projection-down (streams all of x once)
            for c in range(NCH):
                # SWDGE cast-DMA: fp32 HBM -> bf16 SBUF
                x0 = p1x.tile([128, CH], bf16, tag="x0")
                x1 = p1x.tile([128, CH], bf16, tag="x1")
                nc.gpsimd.dma_start(out=x0[:], in_=x_d[0:128, c * CH : (c + 1) * CH])
                nc.gpsimd.dma_start(out=x1[:], in_=x_d[128:256, c * CH : (c + 1) * CH])
                st = p1s.tile([C_MID, CH], bf16, tag="p1st")
                for s in range(NS):
                    pp = psp.tile([C_MID, 512], f32, tag="ps")
                    nc.tensor.matmul(
                        pp[:],
                        w_inT_t[:, 0:C_MID],
                        x0[:, s * 512 : (s + 1) * 512],
                        start=True,
                        stop=False,
                    )
                    nc.tensor.matmul(
                        pp[:],
                        w_inT_t[:, C_MID : 2 * C_MID],
                        x1[:, s * 512 : (s + 1) * 512],
                        start=False,
                        stop=True,
                    )
                    nc.vector.tensor_copy(st[:, s * 512 : (s + 1) * 512], pp[:])
                nc.scalar.dma_start(out=xproj_d[:, c * CH : (c + 1) * CH], in_=st[:])

            # ---- phase 2: FFT -> gain -> IFFT, stage-major, 2 channels per bank
            # bulk one-shot reshape loads: [m, h*w] -> [h, m*128+w]
            Xall = fftb.tile([H, C_MID * H], bf16)
            gall = fftb.tile([H, C_MID * H], bf16)
            nc.sync.dma_start(
                out=Xall[:], in_=xproj_d.ap().rearrange("m (h w) -> h m w", h=H)
            )
            nc.sync.dma_start(
                out=gall[:], in_=gain_d.ap().rearrange("m (h w) -> h m w", h=H)
            )

            sAall = fftb.tile([H, C_MID * 2 * H], bf16)
            sEall = fftb.tile([H, C_MID * 2 * H], bf16)
            sUall = fftb.tile([H, C_MID * 2 * H], bf16)
            sXall = fftb.tile([H, C_MID * H], bf16)
            NP = C_MID // 2  # channel pairs

            for q in range(NP):
                pA = psp.tile([H, 4 * H], f32, tag="ps")
                for j in range(2):
                    Xm = Xall[:, (2 * q + j) * H : (2 * q + j + 1) * H]
                    nc.tensor.matmul(
                        pA[:, 2 * j * H : (2 * j + 2) * H], Xm, Fri[:],
                        start=True, stop=True,
                    )
                nc.vector.tensor_copy(sAall[:, q * 4 * H : (q + 1) * 4 * H], pA[:])

            for q in range(NP):
                pB = psp.tile([H, 4 * H], f32, tag="ps")
                for j in range(2):
                    m = 2 * q + j
                    sAr = sAall[:, m * 2 * H : m * 2 * H + H]
                    sAi = sAall[:, m * 2 * H + H : (m + 1) * 2 * H]
                    bri = pB[:, 2 * j * H : (2 * j + 2) * H]
                    nc.tensor.matmul(bri, sAr, Fri[:], start=True, stop=False)
                    nc.tensor.matmul(bri, sAi, Fnr[:], start=False, stop=True)
                gq = (
                    gall[:, 2 * q * H : (2 * q + 2) * H]
                    .rearrange("p (m w) -> p m w", m=2)
                    .unsqueeze(2)
                    .broadcast_to((H, 2, 2, H))
                )
                nc.vector.tensor_mul(
                    sEall[:, q * 4 * H : (q + 1) * 4 * H].rearrange(
                        "p (m r w) -> p m r w", m=2, r=2
                    ),
                    pB[:].rearrange("p (m r w) -> p m r w", m=2, r=2),
                    gq,
                )

            for q in range(NP):
                pC = psp.tile([H, 4 * H], f32, tag="ps")
                for j in range(2):
                    m = 2 * q + j
                    sEr = sEall[:, m * 2 * H : m * 2 * H + H]
                    sEi = sEall[:, m * 2 * H + H : (m + 1) * 2 * H]
                    cri = pC[:, 2 * j * H : (2 * j + 2) * H]
                    nc.tensor.matmul(cri, sEr, Frn[:], start=True, stop=False)
                    nc.tensor.matmul(cri, sEi, Fir[:], start=False, stop=True)
                nc.vector.tensor_copy(sUall[:, q * 4 * H : (q + 1) * 4 * H], pC[:])

            for q in range(NP):
                pD = psp.tile([H, 2 * H], f32, tag="ps")
                for j in range(2):
                    m = 2 * q + j
                    sUr = sUall[:, m * 2 * H : m * 2 * H + H]
                    sUi = sUall[:, m * 2 * H + H : (m + 1) * 2 * H]
                    xe = pD[:, j * H : (j + 1) * H]
                    nc.tensor.matmul(xe, sUr, fr_t[:], start=True, stop=False)
                    nc.tensor.matmul(xe, sUi, fi_t[:], start=False, stop=True)
                nc.vector.tensor_copy(sXall[:, q * 2 * H : (q + 1) * 2 * H], pD[:])
            nc.scalar.dma_start(
                out=xmid_d.ap().rearrange("m (h w) -> h m w", h=H), in_=sXall[:]
            )

            # ---- phase 3: projection-up + residual (streams x again + out)
            # bf16 residual stream via the otherwise-idle SWDGE ring; tapered
            # final chunks so the write drain is short
            chunks3 = [(i * CH, CH) for i in range(NCH - 1)]
            chunks3 += [(7 * CH, 1024), (7 * CH + 1024, 512), (7 * CH + 1536, 512)]
            for off, sz in chunks3:
                xm = p3m.tile([C_MID, CH], bf16, tag="xm")
                nc.sync.dma_start(out=xm[:, 0:sz], in_=xmid_d[:, off : off + sz])
                for half in range(2):
                    xt = p3x.tile([128, CH], bf16, tag=f"xt{half}")
                    nc.gpsimd.dma_start(
                        out=xt[:, 0:sz],
                        in_=x_d[half * 128 : (half + 1) * 128, off : off + sz],
                    )
                    ot = p3o.tile([128, CH], f32, tag=f"ot{half}")
                    for s in range(sz // 512):
                        po = psp.tile([128, 512], f32, tag="ps")
                        nc.tensor.matmul(
                            po[:],
                            w_outT_t[:, half * 128 : (half + 1) * 128],
                            xm[:, s * 512 : (s + 1) * 512],
                            start=True,
                            stop=True,
                        )
                        nc.vector.tensor_add(
                            ot[:, s * 512 : (s + 1) * 512],
                            po[:],
                            xt[:, s * 512 : (s + 1) * 512],
                        )
                    nc.scalar.dma_start(
                        out=out_d[half * 128 : (half + 1) * 128, off : off + sz],
                        in_=ot[:, 0:sz],
                    )

    nc.compile()
    _CACHE["nc"] = nc
    return nc


def _in_maps(x, w_in, w_out, bin_weights):
    import ml_dtypes

    bf = ml_dtypes.bfloat16
    wsel, frb, fib, fingb = _host_constants()
    x = np.ascontiguousarray(x, dtype=np.float32)
    shared = {
        "w_inT": np.ascontiguousarray(w_in.T.astype(bf)),
        "w_outT": np.ascontiguousarray(w_out.T.astype(bf)),
        "binT": np.ascontiguousarray(bin_weights.reshape(C_MID, AR).T.astype(bf)),
        "wsel": wsel,
        "fr": frb,
        "fi": fib,
        "fineg": fingb,
    }
    return [
        {"x": np.ascontiguousarray(x[b].reshape(C_IN, HW)), **shared}
        for b in range(B)
    ]


def _ensure_ntff_hook():
    """The agent image's antenv lacks axon_hooks; recreate it so
    run_bass_kernel_spmd(trace=True) can capture NTFF profiles."""
    import sys
    import types

    import antenv

    if hasattr(antenv, "axon_hooks"):
        return
    mod = types.ModuleType("antenv.axon_hooks")
    holder = [None]
    mod.set_axon_ntff_profile_hook = lambda h: holder.__setitem__(0, h)
    mod.get_axon_ntff_profile_hook = lambda: holder[0]
    sys.modules["antenv.axon_hooks"] = mod
    antenv.axon_hooks = mod
    try:
        from trn_agent_boot.trn_boot import _ntff_profile_via_ctypes

        mod.set_axon_ntff_profile_hook(
            _ntff_profile_via_ctypes("/opt/axon/libaxon_pjrt.so")
        )
    except Exception:
        pass


def run_on_device(x, w_in, w_out, bin_weights, trace=False):
    from concourse.bass_utils import run_bass_kernel_spmd

    if trace:
        _ensure_ntff_hook()
    nc = _build_nc()
    in_maps = _in_maps(
        np.asarray(x), np.asarray(w_in), np.asarray(w_out), np.asarray(bin_weights)
    )
    res = run_bass_kernel_spmd(nc, in_maps, list(range(B)), trace=trace)
    out = np.stack(
        [res.results[b]["out"].reshape(C_IN, H, W) for b in range(B)], axis=0
    )
    return out.astype(np.float32), res


def kernel(x, w_in, w_out, bin_weights):
    out, _ = run_on_device(x, w_in, w_out, bin_weights, trace=False)
    return out


# revision 23
# speedup vs baseline: 1.0433x; 1.0433x over previous
"""Trainium2 Bass kernel for nn_AngleFreqEnhance.

Reference computation (per batch element b):
    x_proj = w_in @ x_b                    # (16, 256) @ (256, 16384)
    Z      = fftshift(fft2(x_proj, ortho))
    enh    = (|Z|+eps) * gain * Z/|Z|      # == gain * Z up to ~1e-8 absolute
    x_enh  = ifft2(ifftshift(enh), ortho).real
    out_b  = x_b + w_out @ x_enh           # (256, 16) @ (16, 16384)

Device formulation:
  * eps term dropped (absolute error <= gain * 1e-8, far below the 2e-2 gate).
  * fftshift/ifftshift folded into the gain map (host-side ifftshift of the
    constant angle/radius selection maps).
  * 2-D FFT/IFFT done as matmuls with the symmetric 128x128 ortho DFT matrix
    F = Fr + i*Fi (bf16 operands, fp32 PSUM accumulate).  Each matmul stage
    contracts the partition axis and transposes the kept axis, so four stages
    (fwd row, fwd col, inv row, inv col) chain with no explicit transposes.
  * gain = bin_weights-flat @ Wsel, where Wsel[(a,r), pix] is the
    host-precomputed constant angle-weight * radius-indicator map
    (shape-derived constants only; bin_weights stays a device input).
  * All matmul operands are bf16 (full-rate TensorE + fast weight load); the
    residual x stream, gain map, PSUM accumulation, and output stay fp32, so
    bf16 rounding only touches the small enhancement term.

Sharding: pure data parallel - batch element b on core b (B=8, 8 cores).
No collectives. Host gathers per-core outputs.
"""

import math

import numpy as np

B, C_IN, C_MID, H, W = 8, 256, 16, 128, 128
HW = H * W
N_ANGLES = 8
N_RADII = 9
AR = N_ANGLES * N_RADII  # 72
CH = 2048                # pixel chunk for the streaming passes
NCH = HW // CH           # 8
NS = CH // 512           # 4 (PSUM-bank sized sub-chunks)

_CACHE = {}


def _build_masks():
    """Bit-faithful replica of the reference _build_masks on CPU jax.

    The reference's `(arctan2 + pi) % pi` lowers to an IEEE-remainder on
    XLA:CPU (range (-pi/2, pi/2]), which zeroes the angle weights over half
    the plane.  Running the identical jnp ops on the identical backend is the
    only safe way to reproduce the oracle.
    """
    import jax
    import jax.numpy as jnp

    EPS = 1e-8
    RADIUS_WIDTH = 8
    OVERLAP_RATIO = 1.5
    with jax.default_device(jax.devices("cpu")[0]):
        cy, cx = H // 2, W // 2
        y = jnp.arange(H, dtype=jnp.float32)[:, None] - cy
        x = jnp.arange(W, dtype=jnp.float32)[None, :] - cx
        r = jnp.sqrt(y * y + x * x)
        theta = (jnp.arctan2(y, x) + math.pi) % math.pi
        n_radii = int(max(cy, cx) // RADIUS_WIDTH) + 1
        radius_idx = jnp.clip(
            jnp.floor(r / RADIUS_WIDTH).astype(jnp.int32), 0, n_radii - 1
        )
        delta = math.pi / N_ANGLES
        half_width = OVERLAP_RATIO * delta / 2.0
        centers = (jnp.arange(N_ANGLES, dtype=jnp.float32) * delta + delta / 2.0)[
            :, None, None
        ]
        dist = jnp.abs(theta[None] - centers)
        w = jnp.clip(1.0 - dist / half_width, 0.0) * (dist < half_width)
        angle_weights = w / (w.sum(axis=0, keepdims=True) + EPS)
        return np.asarray(radius_idx), np.asarray(angle_weights)


def _host_constants():
    if "consts" in _CACHE:
        return _CACHE["consts"]
    import ml_dtypes

    bf = ml_dtypes.bfloat16
    radius_idx, aw = _build_masks()
    maps = aw[:, None] * (
        radius_idx[None, None] == np.arange(N_RADII)[None, :, None, None]
    ).astype(np.float32)
    wsel = (
        np.fft.ifftshift(maps, axes=(-2, -1))
        .reshape(AR, HW)
        .astype(np.float32)
    )
    k = np.arange(H)
    th = 2.0 * np.pi * np.outer(k, k) / H
    fr = (np.cos(th) / math.sqrt(H)).astype(np.float32)
    fi = (-np.sin(th) / math.sqrt(H)).astype(np.float32)
    consts = (
        np.ascontiguousarray(wsel.astype(bf)),
        np.ascontiguousarray(fr.astype(bf)),
        np.ascontiguousarray(fi.astype(bf)),
        np.ascontiguousarray((-fi).astype(bf)),
    )
    _CACHE["consts"] = consts
    return consts


def _build_nc():
    if "nc" in _CACHE:
        return _CACHE["nc"]
    import concourse.bass as bass
    import concourse.bacc as bacc
    import concourse.tile as tile
    from concourse import mybir

    f32 = mybir.dt.float32
    bf16 = mybir.dt.bfloat16
    PSUM = bass.MemorySpace.PSUM

    nc = bacc.Bacc(
        None,
        target_bir_lowering=False,
        debug=False,
        enable_asserts=False,
        num_devices=B,
    )

    x_d = nc.declare_dram_parameter("x", [C_IN, HW], f32, isOutput=False)
    w_inT_d = nc.declare_dram_parameter("w_inT", [C_IN, C_MID], bf16, isOutput=False)
    w_outT_d = nc.declare_dram_parameter("w_outT", [C_MID, C_IN], bf16, isOutput=False)
    binT_d = nc.declare_dram_parameter("binT", [AR, C_MID], bf16, isOutput=False)
    wsel_d = nc.declare_dram_parameter("wsel", [AR, HW], bf16, isOutput=False)
    fr_d = nc.declare_dram_parameter("fr", [H, H], bf16, isOutput=False)
    fi_d = nc.declare_dram_parameter("fi", [H, H], bf16, isOutput=False)
    fineg_d = nc.declare_dram_parameter("fineg", [H, H], bf16, isOutput=False)
    out_d = nc.declare_dram_parameter("out", [C_IN, HW], f32, isOutput=True)

    xproj_d = nc.dram_tensor("xproj_d", [C_MID, HW], bf16)
    xmid_d = nc.dram_tensor("xmid_d", [C_MID, HW], bf16)
    gain_d = nc.dram_tensor("gain_d", [C_MID, HW], bf16)

    xproj_r = xproj_d.ap().rearrange("m (h w) -> m h w", h=H)
    xmid_r = xmid_d.ap().rearrange("m (h w) -> m h w", h=H)
    gain_r = gain_d.ap().rearrange("m (h w) -> m h w", h=H)

    with tile.TileContext(nc) as tc:
        with (
            tc.tile_pool(name="const", bufs=1) as cpool,
            tc.tile_pool(name="gw", bufs=2) as gwp,
            tc.tile_pool(name="p1x", bufs=3) as p1x,
            tc.tile_pool(name="p1s", bufs=2) as p1s,
            tc.tile_pool(name="fftb", bufs=1) as fftb,
            tc.tile_pool(name="p3m", bufs=2) as p3m,
            tc.tile_pool(name="p3x", bufs=5) as p3x,
            tc.tile_pool(name="p3o", bufs=2) as p3o,
            tc.tile_pool(name="ps", bufs=8, space=PSUM) as psp,
        ):
            # ---- constants into SBUF
            w_inT_t = cpool.tile([128, 2 * C_MID], bf16)
            nc.sync.dma_start(out=w_inT_t[:, 0:C_MID], in_=w_inT_d[0:128, :])
            nc.sync.dma_start(out=w_inT_t[:, C_MID : 2 * C_MID], in_=w_inT_d[128:256, :])
            w_outT_t = cpool.tile([C_MID, C_IN], bf16)
            nc.sync.dma_start(out=w_outT_t[:], in_=w_outT_d[:])
            binT_t = cpool.tile([AR, C_MID], bf16)
            nc.sync.dma_start(out=binT_t[:], in_=binT_d[:])
            fr_t = cpool.tile([H, H], bf16)
            nc.sync.dma_start(out=fr_t[:], in_=fr_d[:])
            fi_t = cpool.tile([H, H], bf16)
            nc.sync.dma_start(out=fi_t[:], in_=fi_d[:])
            fng_t = cpool.tile([H, H], bf16)
            nc.sync.dma_start(out=fng_t[:], in_=fineg_d[:])
            # paired moving operands for batched FFT matmuls
            Fri = cpool.tile([H, 2 * H], bf16)   # [Fr | Fi]
            nc.sync.dma_start(out=Fri[:, 0:H], in_=fr_d[:])
            nc.sync.dma_start(out=Fri[:, H : 2 * H], in_=fi_d[:])
            Fnr = cpool.tile([H, 2 * H], bf16)   # [-Fi | Fr]
            nc.sync.dma_start(out=Fnr[:, 0:H], in_=fineg_d[:])
            nc.sync.dma_start(out=Fnr[:, H : 2 * H], in_=fr_d[:])
            Frn = cpool.tile([H, 2 * H], bf16)   # [Fr | -Fi]
            nc.sync.dma_start(out=Frn[:, 0:H], in_=fr_d[:])
            nc.sync.dma_start(out=Frn[:, H : 2 * H], in_=fineg_d[:])
            Fir = cpool.tile([H, 2 * H], bf16)   # [Fi | Fr]
            nc.sync.dma_start(out=Fir[:, 0:H], in_=fi_d[:])
            nc.sync.dma_start(out=Fir[:, H : 2 * H], in_=fr_d[:])

            # ---- phase 1: projection-down (streams all of x once)
            for c in range(NCH):
                # half the stream as SWDGE cast-DMA, half HWDGE f32 + DVE cast
                x0 = p1x.tile([128, CH], bf16, tag="x0")
                x1f = p1x.tile([128, CH], f32, tag="x1f")
                x1 = p1x.tile([128, CH], bf16, tag="x1")
                nc.gpsimd.dma_start(out=x0[:], in_=x_d[0:128, c * CH : (c + 1) * CH])
                nc.sync.dma_start(out=x1f[:], in_=x_d[128:256, c * CH : (c + 1) * CH])
                nc.vector.tensor_copy(x1[:], x1f[:])
                st = p1s.tile([C_MID, CH], bf16, tag="p1st")
                for s in range(NS):
                    pp = psp.tile([C_MID, 512], f32, tag="ps")
                    nc.tensor.matmul(
                        pp[:],
                        w_inT_t[:, 0:C_MID],
                        x0[:, s * 512 : (s + 1) * 512],
                        start=True,
                        stop=False,
                    )
                    nc.tensor.matmul(
                        pp[:],
                        w_inT_t[:, C_MID : 2 * C_MID],
                        x1[:, s * 512 : (s + 1) * 512],
                        start=False,
                        stop=True,
                    )
                    nc.vector.tensor_copy(st[:, s * 512 : (s + 1) * 512], pp[:])
                nc.scalar.dma_start(out=xproj_d[:, c * CH : (c + 1) * CH], in_=st[:])

            # ---- gain table (rides the pass1 -> FFT boundary)
            for c in range(NCH):
                wt = gwp.tile([AR, CH], bf16)
                nc.sync.dma_start(out=wt[:], in_=wsel_d[:, c * CH : (c + 1) * CH])
                gs = p1s.tile([C_MID, CH], bf16, tag="gs")
                for s in range(NS):
                    pg = psp.tile([C_MID, 512], f32, tag="ps")
                    nc.tensor.matmul(
                        pg[:],
                        binT_t[:],
                        wt[:, s * 512 : (s + 1) * 512],
                        start=True,
                        stop=True,
                    )
                    nc.vector.tensor_copy(gs[:, s * 512 : (s + 1) * 512], pg[:])
                nc.scalar.dma_start(out=gain_d[:, c * CH : (c + 1) * CH], in_=gs[:])

            # ---- phase 2: FFT -> gain -> IFFT, stage-major, 2 channels per bank
            # bulk one-shot reshape loads: [m, h*w] -> [h, m*128+w]
            Xall = fftb.tile([H, C_MID * H], bf16)
            gall = fftb.tile([H, C_MID * H], bf16)
            nc.sync.dma_start(
                out=Xall[:], in_=xproj_d.ap().rearrange("m (h w) -> h m w", h=H)
            )
            nc.sync.dma_start(
                out=gall[:], in_=gain_d.ap().rearrange("m (h w) -> h m w", h=H)
            )

            sAall = fftb.tile([H, C_MID * 2 * H], bf16)
            sEall = fftb.tile([H, C_MID * 2 * H], bf16)
            sUall = fftb.tile([H, C_MID * 2 * H], bf16)
            sXall = fftb.tile([H, C_MID * H], bf16)
            NP = C_MID // 2  # channel pairs

            for q in range(NP):
                pA = psp.tile([H, 4 * H], f32, tag="ps")
                for j in range(2):
                    Xm = Xall[:, (2 * q + j) * H : (2 * q + j + 1) * H]
                    nc.tensor.matmul(
                        pA[:, 2 * j * H : (2 * j + 2) * H], Xm, Fri[:],
                        start=True, stop=True,
                    )
                nc.vector.tensor_copy(sAall[:, q * 4 * H : (q + 1) * 4 * H], pA[:])

            for q in range(NP):
                pB = psp.tile([H, 4 * H], f32, tag="ps")
                for j in range(2):
                    m = 2 * q + j
                    sAr = sAall[:, m * 2 * H : m * 2 * H + H]
                    sAi = sAall[:, m * 2 * H + H : (m + 1) * 2 * H]
                    bri = pB[:, 2 * j * H : (2 * j + 2) * H]
                    nc.tensor.matmul(bri, sAr, Fri[:], start=True, stop=False)
                    nc.tensor.matmul(bri, sAi, Fnr[:], start=False, stop=True)
                gq = (
                    gall[:, 2 * q * H : (2 * q + 2) * H]
                    .rearrange("p (m w) -> p m w", m=2)
                    .unsqueeze(2)
                    .broadcast_to((H, 2, 2, H))
                )
                nc.vector.tensor_mul(
                    sEall[:, q * 4 * H : (q + 1) * 4 * H].rearrange(
                        "p (m r w) -> p m r w", m=2, r=2
                    ),
                    pB[:].rearrange("p (m r w) -> p m r w", m=2, r=2),
                    gq,
                )

            for q in range(NP):
                pC = psp.tile([H, 4 * H], f32, tag="ps")
                for j in range(2):
                    m = 2 * q + j
                    sEr = sEall[:, m * 2 * H : m * 2 * H + H]
                    sEi = sEall[:, m * 2 * H + H : (m + 1) * 2 * H]
                    cri = pC[:, 2 * j * H : (2 * j + 2) * H]
                    nc.tensor.matmul(cri, sEr, Frn[:], start=True, stop=False)
                    nc.tensor.matmul(cri, sEi, Fir[:], start=False, stop=True)
                nc.vector.tensor_copy(sUall[:, q * 4 * H : (q + 1) * 4 * H], pC[:])

            for q in range(NP):
                pD = psp.tile([H, 2 * H], f32, tag="ps")
                for j in range(2):
                    m = 2 * q + j
                    sUr = sUall[:, m * 2 * H : m * 2 * H + H]
                    sUi = sUall[:, m * 2 * H + H : (m + 1) * 2 * H]
                    xe = pD[:, j * H : (j + 1) * H]
                    nc.tensor.matmul(xe, sUr, fr_t[:], start=True, stop=False)
                    nc.tensor.matmul(xe, sUi, fi_t[:], start=False, stop=True)
                nc.vector.tensor_copy(sXall[:, q * 2 * H : (q + 1) * 2 * H], pD[:])
            nc.scalar.dma_start(
                out=xmid_d.ap().rearrange("m (h w) -> h m w", h=H), in_=sXall[:]
            )

            # ---- phase 3: projection-up + residual (streams x again + out)
            # bf16 residual stream via the otherwise-idle SWDGE ring; tapered
            # final chunks so the write drain is short
            chunks3 = [(i * CH, CH) for i in range(NCH - 1)]
            chunks3 += [(7 * CH, 1024), (7 * CH + 1024, 512), (7 * CH + 1536, 512)]
            for off, sz in chunks3:
                xm = p3m.tile([C_MID, CH], bf16, tag="xm")
                nc.sync.dma_start(out=xm[:, 0:sz], in_=xmid_d[:, off : off + sz])
                for half in range(2):
                    xt = p3x.tile([128, CH], bf16, tag=f"xt{half}")
                    nc.gpsimd.dma_start(
                        out=xt[:, 0:sz],
                        in_=x_d[half * 128 : (half + 1) * 128, off : off + sz],
                    )
                    ot = p3o.tile([128, CH], f32, tag=f"ot{half}")
                    for s in range(sz // 512):
                        po = psp.tile([128, 512], f32, tag="ps")
                        nc.tensor.matmul(
                            po[:],
                            w_outT_t[:, half * 128 : (half + 1) * 128],
                            xm[:, s * 512 : (s + 1) * 512],
                            start=True,
                            stop=True,
                        )
                        nc.vector.tensor_add(
                            ot[:, s * 512 : (s + 1) * 512],
                            po[:],
                            xt[:, s * 512 : (s + 1) * 512],
                        )
                    nc.scalar.dma_start(
                        out=out_d[half * 128 : (half + 1) * 128, off : off + sz],
                        in_=ot[:, 0:sz],
                    )

    nc.compile()
    _CACHE["nc"] = nc
    return nc


def _in_maps(x, w_in, w_out, bin_weights):
    import ml_dtypes

    bf = ml_dtypes.bfloat16
    wsel, frb, fib, fingb = _host_constants()
    x = np.ascontiguousarray(x, dtype=np.float32)
    shared = {
        "w_inT": np.ascontiguousarray(w_in.T.astype(bf)),
        "w_outT": np.ascontiguousarray(w_out.T.astype(bf)),
        "binT": np.ascontiguousarray(bin_weights.reshape(C_MID, AR).T.astype(bf)),
        "wsel": wsel,
        "fr": frb,
        "fi": fib,
        "fineg": fingb,
    }
    return [
        {"x": np.ascontiguousarray(x[b].reshape(C_IN, HW)), **shared}
        for b in range(B)
    ]


def _ensure_ntff_hook():
    """The agent image's antenv lacks axon_hooks; recreate it so
    run_bass_kernel_spmd(trace=True) can capture NTFF profiles."""
    import sys
    import types

    import antenv

    if hasattr(antenv, "axon_hooks"):
        return
    mod = types.ModuleType("antenv.axon_hooks")
    holder = [None]
    mod.set_axon_ntff_profile_hook = lambda h: holder.__setitem__(0, h)
    mod.get_axon_ntff_profile_hook = lambda: holder[0]
    sys.modules["antenv.axon_hooks"] = mod
    antenv.axon_hooks = mod
    try:
        from trn_agent_boot.trn_boot import _ntff_profile_via_ctypes

        mod.set_axon_ntff_profile_hook(
            _ntff_profile_via_ctypes("/opt/axon/libaxon_pjrt.so")
        )
    except Exception:
        pass


def run_on_device(x, w_in, w_out, bin_weights, trace=False):
    from concourse.bass_utils import run_bass_kernel_spmd

    if trace:
        _ensure_ntff_hook()
    nc = _build_nc()
    in_maps = _in_maps(
        np.asarray(x), np.asarray(w_in), np.asarray(w_out), np.asarray(bin_weights)
    )
    res = run_bass_kernel_spmd(nc, in_maps, list(range(B)), trace=trace)
    out = np.stack(
        [res.results[b]["out"].reshape(C_IN, H, W) for b in range(B)], axis=0
    )
    return out.astype(np.float32), res


def kernel(x, w_in, w_out, bin_weights):
    out, _ = run_on_device(x, w_in, w_out, bin_weights, trace=False)
    return out


# revision 24
# speedup vs baseline: 1.1201x; 1.0736x over previous
"""Trainium2 Bass kernel for nn_AngleFreqEnhance.

Reference computation (per batch element b):
    x_proj = w_in @ x_b                    # (16, 256) @ (256, 16384)
    Z      = fftshift(fft2(x_proj, ortho))
    enh    = (|Z|+eps) * gain * Z/|Z|      # == gain * Z up to ~1e-8 absolute
    x_enh  = ifft2(ifftshift(enh), ortho).real
    out_b  = x_b + w_out @ x_enh           # (256, 16) @ (16, 16384)

Device formulation:
  * eps term dropped (absolute error <= gain * 1e-8, far below the 2e-2 gate).
  * fftshift/ifftshift folded into the gain map (host-side ifftshift of the
    constant angle/radius selection maps).
  * 2-D FFT/IFFT done as matmuls with the symmetric 128x128 ortho DFT matrix
    F = Fr + i*Fi (bf16 operands, fp32 PSUM accumulate).  Each matmul stage
    contracts the partition axis and transposes the kept axis, so four stages
    (fwd row, fwd col, inv row, inv col) chain with no explicit transposes.
  * gain = bin_weights-flat @ Wsel, where Wsel[(a,r), pix] is the
    host-precomputed constant angle-weight * radius-indicator map
    (shape-derived constants only; bin_weights stays a device input).
  * All matmul operands are bf16 (full-rate TensorE + fast weight load); the
    residual x stream, gain map, PSUM accumulation, and output stay fp32, so
    bf16 rounding only touches the small enhancement term.

Sharding: pure data parallel - batch element b on core b (B=8, 8 cores).
No collectives. Host gathers per-core outputs.
"""

import math

import numpy as np

B, C_IN, C_MID, H, W = 8, 256, 16, 128, 128
HW = H * W
N_ANGLES = 8
N_RADII = 9
AR = N_ANGLES * N_RADII  # 72
CH = 2048                # pixel chunk for the streaming passes
NCH = HW // CH           # 8
NS = CH // 512           # 4 (PSUM-bank sized sub-chunks)

_CACHE = {}


def _build_masks():
    """Bit-faithful replica of the reference _build_masks on CPU jax.

    The reference's `(arctan2 + pi) % pi` lowers to an IEEE-remainder on
    XLA:CPU (range (-pi/2, pi/2]), which zeroes the angle weights over half
    the plane.  Running the identical jnp ops on the identical backend is the
    only safe way to reproduce the oracle.
    """
    import jax
    import jax.numpy as jnp

    EPS = 1e-8
    RADIUS_WIDTH = 8
    OVERLAP_RATIO = 1.5
    with jax.default_device(jax.devices("cpu")[0]):
        cy, cx = H // 2, W // 2
        y = jnp.arange(H, dtype=jnp.float32)[:, None] - cy
        x = jnp.arange(W, dtype=jnp.float32)[None, :] - cx
        r = jnp.sqrt(y * y + x * x)
        theta = (jnp.arctan2(y, x) + math.pi) % math.pi
        n_radii = int(max(cy, cx) // RADIUS_WIDTH) + 1
        radius_idx = jnp.clip(
            jnp.floor(r / RADIUS_WIDTH).astype(jnp.int32), 0, n_radii - 1
        )
        delta = math.pi / N_ANGLES
        half_width = OVERLAP_RATIO * delta / 2.0
        centers = (jnp.arange(N_ANGLES, dtype=jnp.float32) * delta + delta / 2.0)[
            :, None, None
        ]
        dist = jnp.abs(theta[None] - centers)
        w = jnp.clip(1.0 - dist / half_width, 0.0) * (dist < half_width)
        angle_weights = w / (w.sum(axis=0, keepdims=True) + EPS)
        return np.asarray(radius_idx), np.asarray(angle_weights)


def _host_constants():
    if "consts" in _CACHE:
        return _CACHE["consts"]
    import ml_dtypes

    bf = ml_dtypes.bfloat16
    radius_idx, aw = _build_masks()
    maps = aw[:, None] * (
        radius_idx[None, None] == np.arange(N_RADII)[None, :, None, None]
    ).astype(np.float32)
    wsel = (
        np.fft.ifftshift(maps, axes=(-2, -1))
        .reshape(AR, HW)
        .astype(np.float32)
    )
    k = np.arange(H)
    th = 2.0 * np.pi * np.outer(k, k) / H
    fr = (np.cos(th) / math.sqrt(H)).astype(np.float32)
    fi = (-np.sin(th) / math.sqrt(H)).astype(np.float32)
    consts = (
        np.ascontiguousarray(wsel.astype(bf)),
        np.ascontiguousarray(fr.astype(bf)),
        np.ascontiguousarray(fi.astype(bf)),
        np.ascontiguousarray((-fi).astype(bf)),
    )
    _CACHE["consts"] = consts
    return consts


def _build_nc():
    if "nc" in _CACHE:
        return _CACHE["nc"]
    import concourse.bass as bass
    import concourse.bacc as bacc
    import concourse.tile as tile
    from concourse import mybir

    f32 = mybir.dt.float32
    bf16 = mybir.dt.bfloat16
    PSUM = bass.MemorySpace.PSUM

    nc = bacc.Bacc(
        None,
        target_bir_lowering=False,
        debug=False,
        enable_asserts=False,
        num_devices=B,
    )

    x_d = nc.declare_dram_parameter("x", [C_IN, HW], f32, isOutput=False)
    w_inT_d = nc.declare_dram_parameter("w_inT", [C_IN, C_MID], bf16, isOutput=False)
    w_outT_d = nc.declare_dram_parameter("w_outT", [C_MID, C_IN], bf16, isOutput=False)
    binT_d = nc.declare_dram_parameter("binT", [AR, C_MID], bf16, isOutput=False)
    wsel_d = nc.declare_dram_parameter("wsel", [AR, HW], bf16, isOutput=False)
    fr_d = nc.declare_dram_parameter("fr", [H, H], bf16, isOutput=False)
    fi_d = nc.declare_dram_parameter("fi", [H, H], bf16, isOutput=False)
    fineg_d = nc.declare_dram_parameter("fineg", [H, H], bf16, isOutput=False)
    out_d = nc.declare_dram_parameter("out", [C_IN, HW], f32, isOutput=True)

    xproj_d = nc.dram_tensor("xproj_d", [C_MID, HW], bf16)
    xmid_d = nc.dram_tensor("xmid_d", [C_MID, HW], bf16)
    gain_d = nc.dram_tensor("gain_d", [C_MID, HW], bf16)

    xproj_r = xproj_d.ap().rearrange("m (h w) -> m h w", h=H)
    xmid_r = xmid_d.ap().rearrange("m (h w) -> m h w", h=H)
    gain_r = gain_d.ap().rearrange("m (h w) -> m h w", h=H)

    with tile.TileContext(nc) as tc:
        with (
            tc.tile_pool(name="const", bufs=1) as cpool,
            tc.tile_pool(name="gw", bufs=2) as gwp,
            tc.tile_pool(name="p1x", bufs=3) as p1x,
            tc.tile_pool(name="p1s", bufs=2) as p1s,
            tc.tile_pool(name="fftb", bufs=1) as fftb,
            tc.tile_pool(name="p3m", bufs=2) as p3m,
            tc.tile_pool(name="p3x", bufs=6) as p3x,
            tc.tile_pool(name="p3o", bufs=2) as p3o,
            tc.tile_pool(name="ps", bufs=8, space=PSUM) as psp,
        ):
            # ---- constants into SBUF
            w_inT_t = cpool.tile([128, 2 * C_MID], bf16)
            nc.sync.dma_start(out=w_inT_t[:, 0:C_MID], in_=w_inT_d[0:128, :])
            nc.sync.dma_start(out=w_inT_t[:, C_MID : 2 * C_MID], in_=w_inT_d[128:256, :])
            w_outT_t = cpool.tile([C_MID, C_IN], bf16)
            nc.sync.dma_start(out=w_outT_t[:], in_=w_outT_d[:])
            binT_t = cpool.tile([AR, C_MID], bf16)
            nc.sync.dma_start(out=binT_t[:], in_=binT_d[:])
            fr_t = cpool.tile([H, H], bf16)
            nc.sync.dma_start(out=fr_t[:], in_=fr_d[:])
            fi_t = cpool.tile([H, H], bf16)
            nc.sync.dma_start(out=fi_t[:], in_=fi_d[:])
            fng_t = cpool.tile([H, H], bf16)
            nc.sync.dma_start(out=fng_t[:], in_=fineg_d[:])
            # paired moving operands for batched FFT matmuls
            Fri = cpool.tile([H, 2 * H], bf16)   # [Fr | Fi]
            nc.sync.dma_start(out=Fri[:, 0:H], in_=fr_d[:])
            nc.sync.dma_start(out=Fri[:, H : 2 * H], in_=fi_d[:])
            Fnr = cpool.tile([H, 2 * H], bf16)   # [-Fi | Fr]
            nc.sync.dma_start(out=Fnr[:, 0:H], in_=fineg_d[:])
            nc.sync.dma_start(out=Fnr[:, H : 2 * H], in_=fr_d[:])
            Frn = cpool.tile([H, 2 * H], bf16)   # [Fr | -Fi]
            nc.sync.dma_start(out=Frn[:, 0:H], in_=fr_d[:])
            nc.sync.dma_start(out=Frn[:, H : 2 * H], in_=fineg_d[:])
            Fir = cpool.tile([H, 2 * H], bf16)   # [Fi | Fr]
            nc.sync.dma_start(out=Fir[:, 0:H], in_=fi_d[:])
            nc.sync.dma_start(out=Fir[:, H : 2 * H], in_=fr_d[:])

            # ---- phase 1: projection-down (streams all of x once)
            for c in range(NCH):
                # SWDGE cast-DMA: fp32 HBM -> bf16 SBUF
                x0 = p1x.tile([128, CH], bf16, tag="x0")
                x1 = p1x.tile([128, CH], bf16, tag="x1")
                nc.gpsimd.dma_start(out=x0[:], in_=x_d[0:128, c * CH : (c + 1) * CH])
                nc.gpsimd.dma_start(out=x1[:], in_=x_d[128:256, c * CH : (c + 1) * CH])
                st = p1s.tile([C_MID, CH], bf16, tag="p1st")
                for s in range(NS):
                    pp = psp.tile([C_MID, 512], f32, tag="ps")
                    nc.tensor.matmul(
                        pp[:],
                        w_inT_t[:, 0:C_MID],
                        x0[:, s * 512 : (s + 1) * 512],
                        start=True,
                        stop=False,
                    )
                    nc.tensor.matmul(
                        pp[:],
                        w_inT_t[:, C_MID : 2 * C_MID],
                        x1[:, s * 512 : (s + 1) * 512],
                        start=False,
                        stop=True,
                    )
                    nc.vector.tensor_copy(st[:, s * 512 : (s + 1) * 512], pp[:])
                nc.scalar.dma_start(out=xproj_d[:, c * CH : (c + 1) * CH], in_=st[:])

            # ---- gain table (rides the pass1 -> FFT boundary)
            for c in range(NCH):
                wt = gwp.tile([AR, CH], bf16)
                nc.sync.dma_start(out=wt[:], in_=wsel_d[:, c * CH : (c + 1) * CH])
                gs = p1s.tile([C_MID, CH], bf16, tag="gs")
                for s in range(NS):
                    pg = psp.tile([C_MID, 512], f32, tag="ps")
                    nc.tensor.matmul(
                        pg[:],
                        binT_t[:],
                        wt[:, s * 512 : (s + 1) * 512],
                        start=True,
                        stop=True,
                    )
                    nc.vector.tensor_copy(gs[:, s * 512 : (s + 1) * 512], pg[:])
                nc.scalar.dma_start(out=gain_d[:, c * CH : (c + 1) * CH], in_=gs[:])

            # ---- phase 2: FFT -> gain -> IFFT, stage-major, 2 channels per bank
            # bulk one-shot reshape loads: [m, h*w] -> [h, m*128+w]
            Xall = fftb.tile([H, C_MID * H], bf16)
            gall = fftb.tile([H, C_MID * H], bf16)
            nc.sync.dma_start(
                out=Xall[:], in_=xproj_d.ap().rearrange("m (h w) -> h m w", h=H)
            )
            nc.sync.dma_start(
                out=gall[:], in_=gain_d.ap().rearrange("m (h w) -> h m w", h=H)
            )

            sAall = fftb.tile([H, C_MID * 2 * H], bf16)
            sEall = fftb.tile([H, C_MID * 2 * H], bf16)
            sUall = fftb.tile([H, C_MID * 2 * H], bf16)
            sXall = fftb.tile([H, C_MID * H], bf16)
            NP = C_MID // 2  # channel pairs

            for q in range(NP):
                pA = psp.tile([H, 4 * H], f32, tag="ps")
                for j in range(2):
                    Xm = Xall[:, (2 * q + j) * H : (2 * q + j + 1) * H]
                    nc.tensor.matmul(
                        pA[:, 2 * j * H : (2 * j + 2) * H], Xm, Fri[:],
                        start=True, stop=True,
                    )
                nc.vector.tensor_copy(sAall[:, q * 4 * H : (q + 1) * 4 * H], pA[:])

            for q in range(NP):
                pB = psp.tile([H, 4 * H], f32, tag="ps")
                for j in range(2):
                    m = 2 * q + j
                    sAr = sAall[:, m * 2 * H : m * 2 * H + H]
                    sAi = sAall[:, m * 2 * H + H : (m + 1) * 2 * H]
                    bri = pB[:, 2 * j * H : (2 * j + 2) * H]
                    nc.tensor.matmul(bri, sAr, Fri[:], start=True, stop=False)
                    nc.tensor.matmul(bri, sAi, Fnr[:], start=False, stop=True)
                gq = (
                    gall[:, 2 * q * H : (2 * q + 2) * H]
                    .rearrange("p (m w) -> p m w", m=2)
                    .unsqueeze(2)
                    .broadcast_to((H, 2, 2, H))
                )
                nc.vector.tensor_mul(
                    sEall[:, q * 4 * H : (q + 1) * 4 * H].rearrange(
                        "p (m r w) -> p m r w", m=2, r=2
                    ),
                    pB[:].rearrange("p (m r w) -> p m r w", m=2, r=2),
                    gq,
                )

            for q in range(NP):
                pC = psp.tile([H, 4 * H], f32, tag="ps")
                for j in range(2):
                    m = 2 * q + j
                    sEr = sEall[:, m * 2 * H : m * 2 * H + H]
                    sEi = sEall[:, m * 2 * H + H : (m + 1) * 2 * H]
                    cri = pC[:, 2 * j * H : (2 * j + 2) * H]
                    nc.tensor.matmul(cri, sEr, Frn[:], start=True, stop=False)
                    nc.tensor.matmul(cri, sEi, Fir[:], start=False, stop=True)
                nc.vector.tensor_copy(sUall[:, q * 4 * H : (q + 1) * 4 * H], pC[:])

            for q in range(NP):
                pD = psp.tile([H, 2 * H], f32, tag="ps")
                for j in range(2):
                    m = 2 * q + j
                    sUr = sUall[:, m * 2 * H : m * 2 * H + H]
                    sUi = sUall[:, m * 2 * H + H : (m + 1) * 2 * H]
                    xe = pD[:, j * H : (j + 1) * H]
                    nc.tensor.matmul(xe, sUr, fr_t[:], start=True, stop=False)
                    nc.tensor.matmul(xe, sUi, fi_t[:], start=False, stop=True)
                nc.vector.tensor_copy(sXall[:, q * 2 * H : (q + 1) * 2 * H], pD[:])
            nc.scalar.dma_start(
                out=xmid_d.ap().rearrange("m (h w) -> h m w", h=H), in_=sXall[:]
            )

            # ---- phase 3: projection-up + residual (streams x again + out)
            # bf16 residual stream via the otherwise-idle SWDGE ring; tapered
            # final chunks so the write drain is short
            chunks3 = [(i * CH, CH) for i in range(NCH - 1)]
            chunks3 += [(7 * CH, 1024), (7 * CH + 1024, 512), (7 * CH + 1536, 512)]
            for off, sz in chunks3:
                xm = p3m.tile([C_MID, CH], bf16, tag="xm")
                nc.sync.dma_start(out=xm[:, 0:sz], in_=xmid_d[:, off : off + sz])
                for half in range(2):
                    xt = p3x.tile([128, CH], bf16, tag=f"xt{half}")
                    nc.gpsimd.dma_start(
                        out=xt[:, 0:sz],
                        in_=x_d[half * 128 : (half + 1) * 128, off : off + sz],
                    )
                    ot = p3o.tile([128, CH], f32, tag=f"ot{half}")
                    for s in range(sz // 512):
                        po = psp.tile([128, 512], f32, tag="ps")
                        nc.tensor.matmul(
                            po[:],
                            w_outT_t[:, half * 128 : (half + 1) * 128],
                            xm[:, s * 512 : (s + 1) * 512],
                            start=True,
                            stop=True,
                        )
                        nc.vector.tensor_add(
                            ot[:, s * 512 : (s + 1) * 512],
                            po[:],
                            xt[:, s * 512 : (s + 1) * 512],
                        )
                    nc.scalar.dma_start(
                        out=out_d[half * 128 : (half + 1) * 128, off : off + sz],
                        in_=ot[:, 0:sz],
                    )

    nc.compile()
    _CACHE["nc"] = nc
    return nc


def _in_maps(x, w_in, w_out, bin_weights):
    import ml_dtypes

    bf = ml_dtypes.bfloat16
    wsel, frb, fib, fingb = _host_constants()
    x = np.ascontiguousarray(x, dtype=np.float32)
    shared = {
        "w_inT": np.ascontiguousarray(w_in.T.astype(bf)),
        "w_outT": np.ascontiguousarray(w_out.T.astype(bf)),
        "binT": np.ascontiguousarray(bin_weights.reshape(C_MID, AR).T.astype(bf)),
        "wsel": wsel,
        "fr": frb,
        "fi": fib,
        "fineg": fingb,
    }
    return [
        {"x": np.ascontiguousarray(x[b].reshape(C_IN, HW)), **shared}
        for b in range(B)
    ]


def _ensure_ntff_hook():
    """The agent image's antenv lacks axon_hooks; recreate it so
    run_bass_kernel_spmd(trace=True) can capture NTFF profiles."""
    import sys
    import types

    import antenv

    if hasattr(antenv, "axon_hooks"):
        return
    mod = types.ModuleType("antenv.axon_hooks")
    holder = [None]
    mod.set_axon_ntff_profile_hook = lambda h: holder.__setitem__(0, h)
    mod.get_axon_ntff_profile_hook = lambda: holder[0]
    sys.modules["antenv.axon_hooks"] = mod
    antenv.axon_hooks = mod
    try:
        from trn_agent_boot.trn_boot import _ntff_profile_via_ctypes

        mod.set_axon_ntff_profile_hook(
            _ntff_profile_via_ctypes("/opt/axon/libaxon_pjrt.so")
        )
    except Exception:
        pass


def run_on_device(x, w_in, w_out, bin_weights, trace=False):
    from concourse.bass_utils import run_bass_kernel_spmd

    if trace:
        _ensure_ntff_hook()
    nc = _build_nc()
    in_maps = _in_maps(
        np.asarray(x), np.asarray(w_in), np.asarray(w_out), np.asarray(bin_weights)
    )
    res = run_bass_kernel_spmd(nc, in_maps, list(range(B)), trace=trace)
    out = np.stack(
        [res.results[b]["out"].reshape(C_IN, H, W) for b in range(B)], axis=0
    )
    return out.astype(np.float32), res


def kernel(x, w_in, w_out, bin_weights):
    out, _ = run_on_device(x, w_in, w_out, bin_weights, trace=False)
    return out


# revision 25
# speedup vs baseline: 1.1377x; 1.0157x over previous
"""Trainium2 Bass kernel for nn_AngleFreqEnhance.

Reference computation (per batch element b):
    x_proj = w_in @ x_b                    # (16, 256) @ (256, 16384)
    Z      = fftshift(fft2(x_proj, ortho))
    enh    = (|Z|+eps) * gain * Z/|Z|      # == gain * Z up to ~1e-8 absolute
    x_enh  = ifft2(ifftshift(enh), ortho).real
    out_b  = x_b + w_out @ x_enh           # (256, 16) @ (16, 16384)

Device formulation:
  * eps term dropped (absolute error <= gain * 1e-8, far below the 2e-2 gate).
  * fftshift/ifftshift folded into the gain map (host-side ifftshift of the
    constant angle/radius selection maps).
  * 2-D FFT/IFFT done as matmuls with the symmetric 128x128 ortho DFT matrix
    F = Fr + i*Fi (bf16 operands, fp32 PSUM accumulate).  Each matmul stage
    contracts the partition axis and transposes the kept axis, so four stages
    (fwd row, fwd col, inv row, inv col) chain with no explicit transposes.
  * gain = bin_weights-flat @ Wsel, where Wsel[(a,r), pix] is the
    host-precomputed constant angle-weight * radius-indicator map
    (shape-derived constants only; bin_weights stays a device input).
  * All matmul operands are bf16 (full-rate TensorE + fast weight load); the
    residual x stream, gain map, PSUM accumulation, and output stay fp32, so
    bf16 rounding only touches the small enhancement term.

Sharding: pure data parallel - batch element b on core b (B=8, 8 cores).
No collectives. Host gathers per-core outputs.
"""

import math

import numpy as np

B, C_IN, C_MID, H, W = 8, 256, 16, 128, 128
HW = H * W
N_ANGLES = 8
N_RADII = 9
AR = N_ANGLES * N_RADII  # 72
CH = 2048                # pixel chunk for the streaming passes
NCH = HW // CH           # 8
NS = CH // 512           # 4 (PSUM-bank sized sub-chunks)

_CACHE = {}


def _build_masks():
    """Bit-faithful replica of the reference _build_masks on CPU jax.

    The reference's `(arctan2 + pi) % pi` lowers to an IEEE-remainder on
    XLA:CPU (range (-pi/2, pi/2]), which zeroes the angle weights over half
    the plane.  Running the identical jnp ops on the identical backend is the
    only safe way to reproduce the oracle.
    """
    import jax
    import jax.numpy as jnp

    EPS = 1e-8
    RADIUS_WIDTH = 8
    OVERLAP_RATIO = 1.5
    with jax.default_device(jax.devices("cpu")[0]):
        cy, cx = H // 2, W // 2
        y = jnp.arange(H, dtype=jnp.float32)[:, None] - cy
        x = jnp.arange(W, dtype=jnp.float32)[None, :] - cx
        r = jnp.sqrt(y * y + x * x)
        theta = (jnp.arctan2(y, x) + math.pi) % math.pi
        n_radii = int(max(cy, cx) // RADIUS_WIDTH) + 1
        radius_idx = jnp.clip(
            jnp.floor(r / RADIUS_WIDTH).astype(jnp.int32), 0, n_radii - 1
        )
        delta = math.pi / N_ANGLES
        half_width = OVERLAP_RATIO * delta / 2.0
        centers = (jnp.arange(N_ANGLES, dtype=jnp.float32) * delta + delta / 2.0)[
            :, None, None
        ]
        dist = jnp.abs(theta[None] - centers)
        w = jnp.clip(1.0 - dist / half_width, 0.0) * (dist < half_width)
        angle_weights = w / (w.sum(axis=0, keepdims=True) + EPS)
        return np.asarray(radius_idx), np.asarray(angle_weights)


def _host_constants():
    if "consts" in _CACHE:
        return _CACHE["consts"]
    import ml_dtypes

    bf = ml_dtypes.bfloat16
    radius_idx, aw = _build_masks()
    maps = aw[:, None] * (
        radius_idx[None, None] == np.arange(N_RADII)[None, :, None, None]
    ).astype(np.float32)
    wsel = (
        np.fft.ifftshift(maps, axes=(-2, -1))
        .reshape(AR, HW)
        .astype(np.float32)
    )
    k = np.arange(H)
    th = 2.0 * np.pi * np.outer(k, k) / H
    fr = (np.cos(th) / math.sqrt(H)).astype(np.float32)
    fi = (-np.sin(th) / math.sqrt(H)).astype(np.float32)
    consts = (
        np.ascontiguousarray(wsel.astype(bf)),
        np.ascontiguousarray(fr.astype(bf)),
        np.ascontiguousarray(fi.astype(bf)),
        np.ascontiguousarray((-fi).astype(bf)),
    )
    _CACHE["consts"] = consts
    return consts


def _build_nc():
    if "nc" in _CACHE:
        return _CACHE["nc"]
    import concourse.bass as bass
    import concourse.bacc as bacc
    import concourse.tile as tile
    from concourse import mybir

    f32 = mybir.dt.float32
    bf16 = mybir.dt.bfloat16
    PSUM = bass.MemorySpace.PSUM

    nc = bacc.Bacc(
        None,
        target_bir_lowering=False,
        debug=False,
        enable_asserts=False,
        num_devices=B,
    )

    x_d = nc.declare_dram_parameter("x", [C_IN, HW], f32, isOutput=False)
    w_inT_d = nc.declare_dram_parameter("w_inT", [C_IN, C_MID], bf16, isOutput=False)
    w_outT_d = nc.declare_dram_parameter("w_outT", [C_MID, C_IN], bf16, isOutput=False)
    binT_d = nc.declare_dram_parameter("binT", [AR, C_MID], bf16, isOutput=False)
    wsel_d = nc.declare_dram_parameter("wsel", [AR, HW], bf16, isOutput=False)
    fr_d = nc.declare_dram_parameter("fr", [H, H], bf16, isOutput=False)
    fi_d = nc.declare_dram_parameter("fi", [H, H], bf16, isOutput=False)
    fineg_d = nc.declare_dram_parameter("fineg", [H, H], bf16, isOutput=False)
    out_d = nc.declare_dram_parameter("out", [C_IN, HW], f32, isOutput=True)

    xproj_d = nc.dram_tensor("xproj_d", [C_MID, HW], bf16)
    xmid_d = nc.dram_tensor("xmid_d", [C_MID, HW], bf16)
    gain_d = nc.dram_tensor("gain_d", [C_MID, HW], bf16)

    xproj_r = xproj_d.ap().rearrange("m (h w) -> m h w", h=H)
    xmid_r = xmid_d.ap().rearrange("m (h w) -> m h w", h=H)
    gain_r = gain_d.ap().rearrange("m (h w) -> m h w", h=H)

    with tile.TileContext(nc) as tc:
        with (
            tc.tile_pool(name="const", bufs=1) as cpool,
            tc.tile_pool(name="gw", bufs=2) as gwp,
            tc.tile_pool(name="p1x", bufs=3) as p1x,
            tc.tile_pool(name="p1s", bufs=2) as p1s,
            tc.tile_pool(name="fftb", bufs=1) as fftb,
            tc.tile_pool(name="p3m", bufs=2) as p3m,
            tc.tile_pool(name="p3x", bufs=6) as p3x,
            tc.tile_pool(name="p3o", bufs=2) as p3o,
            tc.tile_pool(name="ps", bufs=8, space=PSUM) as psp,
        ):
            # ---- constants into SBUF
            w_inT_t = cpool.tile([128, 2 * C_MID], bf16)
            nc.sync.dma_start(out=w_inT_t[:, 0:C_MID], in_=w_inT_d[0:128, :])
            nc.sync.dma_start(out=w_inT_t[:, C_MID : 2 * C_MID], in_=w_inT_d[128:256, :])
            w_outT_t = cpool.tile([C_MID, C_IN], bf16)
            nc.sync.dma_start(out=w_outT_t[:], in_=w_outT_d[:])
            binT_t = cpool.tile([AR, C_MID], bf16)
            nc.sync.dma_start(out=binT_t[:], in_=binT_d[:])
            fr_t = cpool.tile([H, H], bf16)
            nc.sync.dma_start(out=fr_t[:], in_=fr_d[:])
            fi_t = cpool.tile([H, H], bf16)
            nc.sync.dma_start(out=fi_t[:], in_=fi_d[:])
            fng_t = cpool.tile([H, H], bf16)
            nc.sync.dma_start(out=fng_t[:], in_=fineg_d[:])
            # paired moving operands for batched FFT matmuls
            Fri = cpool.tile([H, 2 * H], bf16)   # [Fr | Fi]
            nc.sync.dma_start(out=Fri[:, 0:H], in_=fr_d[:])
            nc.sync.dma_start(out=Fri[:, H : 2 * H], in_=fi_d[:])
            Fnr = cpool.tile([H, 2 * H], bf16)   # [-Fi | Fr]
            nc.sync.dma_start(out=Fnr[:, 0:H], in_=fineg_d[:])
            nc.sync.dma_start(out=Fnr[:, H : 2 * H], in_=fr_d[:])
            Frn = cpool.tile([H, 2 * H], bf16)   # [Fr | -Fi]
            nc.sync.dma_start(out=Frn[:, 0:H], in_=fr_d[:])
            nc.sync.dma_start(out=Frn[:, H : 2 * H], in_=fineg_d[:])
            Fir = cpool.tile([H, 2 * H], bf16)   # [Fi | Fr]
            nc.sync.dma_start(out=Fir[:, 0:H], in_=fi_d[:])
            nc.sync.dma_start(out=Fir[:, H : 2 * H], in_=fr_d[:])

            # ---- phase 1: projection-down (streams all of x once)
            for c in range(NCH):
                # SWDGE cast-DMA: fp32 HBM -> bf16 SBUF
                x0 = p1x.tile([128, CH], bf16, tag="x0")
                x1 = p1x.tile([128, CH], bf16, tag="x1")
                nc.gpsimd.dma_start(out=x0[:], in_=x_d[0:128, c * CH : (c + 1) * CH])
                nc.gpsimd.dma_start(out=x1[:], in_=x_d[128:256, c * CH : (c + 1) * CH])
                st = p1s.tile([C_MID, CH], bf16, tag="p1st")
                for s in range(NS):
                    pp = psp.tile([C_MID, 512], f32, tag="ps")
                    nc.tensor.matmul(
                        pp[:],
                        w_inT_t[:, 0:C_MID],
                        x0[:, s * 512 : (s + 1) * 512],
                        start=True,
                        stop=False,
                    )
                    nc.tensor.matmul(
                        pp[:],
                        w_inT_t[:, C_MID : 2 * C_MID],
                        x1[:, s * 512 : (s + 1) * 512],
                        start=False,
                        stop=True,
                    )
                    nc.vector.tensor_copy(st[:, s * 512 : (s + 1) * 512], pp[:])
                nc.scalar.dma_start(out=xproj_d[:, c * CH : (c + 1) * CH], in_=st[:])

            # ---- gain table (rides the pass1 -> FFT boundary)
            for c in range(NCH):
                wt = gwp.tile([AR, CH], bf16)
                nc.sync.dma_start(out=wt[:], in_=wsel_d[:, c * CH : (c + 1) * CH])
                gs = p1s.tile([C_MID, CH], bf16, tag="gs")
                for s in range(NS):
                    pg = psp.tile([C_MID, 512], f32, tag="ps")
                    nc.tensor.matmul(
                        pg[:],
                        binT_t[:],
                        wt[:, s * 512 : (s + 1) * 512],
                        start=True,
                        stop=True,
                    )
                    nc.scalar.copy(gs[:, s * 512 : (s + 1) * 512], pg[:])
                nc.scalar.dma_start(out=gain_d[:, c * CH : (c + 1) * CH], in_=gs[:])

            # ---- phase 2: FFT -> gain -> IFFT, stage-major, 2 channels per bank
            # bulk one-shot reshape loads: [m, h*w] -> [h, m*128+w]
            Xall = fftb.tile([H, C_MID * H], bf16)
            gall = fftb.tile([H, C_MID * H], bf16)
            nc.sync.dma_start(
                out=Xall[:], in_=xproj_d.ap().rearrange("m (h w) -> h m w", h=H)
            )
            nc.sync.dma_start(
                out=gall[:], in_=gain_d.ap().rearrange("m (h w) -> h m w", h=H)
            )

            sAall = fftb.tile([H, C_MID * 2 * H], bf16)
            sEall = fftb.tile([H, C_MID * 2 * H], bf16)
            sUall = fftb.tile([H, C_MID * 2 * H], bf16)
            sXall = fftb.tile([H, C_MID * H], bf16)
            NP = C_MID // 2  # channel pairs

            for q in range(NP):
                pA = psp.tile([H, 4 * H], f32, tag="ps")
                for j in range(2):
                    Xm = Xall[:, (2 * q + j) * H : (2 * q + j + 1) * H]
                    nc.tensor.matmul(
                        pA[:, 2 * j * H : (2 * j + 2) * H], Xm, Fri[:],
                        start=True, stop=True,
                    )
                engA = nc.scalar.copy if q % 4 == 3 else nc.vector.tensor_copy
                engA(sAall[:, q * 4 * H : (q + 1) * 4 * H], pA[:])

            for q in range(NP):
                pB = psp.tile([H, 4 * H], f32, tag="ps")
                for j in range(2):
                    m = 2 * q + j
                    sAr = sAall[:, m * 2 * H : m * 2 * H + H]
                    sAi = sAall[:, m * 2 * H + H : (m + 1) * 2 * H]
                    bri = pB[:, 2 * j * H : (2 * j + 2) * H]
                    nc.tensor.matmul(bri, sAr, Fri[:], start=True, stop=False)
                    nc.tensor.matmul(bri, sAi, Fnr[:], start=False, stop=True)
                gq = (
                    gall[:, 2 * q * H : (2 * q + 2) * H]
                    .rearrange("p (m w) -> p m w", m=2)
                    .unsqueeze(2)
                    .broadcast_to((H, 2, 2, H))
                )
                nc.vector.tensor_mul(
                    sEall[:, q * 4 * H : (q + 1) * 4 * H].rearrange(
                        "p (m r w) -> p m r w", m=2, r=2
                    ),
                    pB[:].rearrange("p (m r w) -> p m r w", m=2, r=2),
                    gq,
                )

            for q in range(NP):
                pC = psp.tile([H, 4 * H], f32, tag="ps")
                for j in range(2):
                    m = 2 * q + j
                    sEr = sEall[:, m * 2 * H : m * 2 * H + H]
                    sEi = sEall[:, m * 2 * H + H : (m + 1) * 2 * H]
                    cri = pC[:, 2 * j * H : (2 * j + 2) * H]
                    nc.tensor.matmul(cri, sEr, Frn[:], start=True, stop=False)
                    nc.tensor.matmul(cri, sEi, Fir[:], start=False, stop=True)
                engC = nc.scalar.copy if q % 4 == 3 else nc.vector.tensor_copy
                engC(sUall[:, q * 4 * H : (q + 1) * 4 * H], pC[:])

            for q in range(NP):
                pD = psp.tile([H, 2 * H], f32, tag="ps")
                for j in range(2):
                    m = 2 * q + j
                    sUr = sUall[:, m * 2 * H : m * 2 * H + H]
                    sUi = sUall[:, m * 2 * H + H : (m + 1) * 2 * H]
                    xe = pD[:, j * H : (j + 1) * H]
                    nc.tensor.matmul(xe, sUr, fr_t[:], start=True, stop=False)
                    nc.tensor.matmul(xe, sUi, fi_t[:], start=False, stop=True)
                engD = nc.scalar.copy if q % 4 == 3 else nc.vector.tensor_copy
                engD(sXall[:, q * 2 * H : (q + 1) * 2 * H], pD[:])
                nc.scalar.dma_start(
                    out=xmid_d.ap().rearrange("m (h w) -> h m w", h=H)[
                        :, 2 * q : 2 * q + 2, :
                    ],
                    in_=sXall[:, q * 2 * H : (q + 1) * 2 * H],
                )

            # ---- phase 3: projection-up + residual (streams x again + out)
            # bf16 residual stream via the otherwise-idle SWDGE ring; tapered
            # final chunks so the write drain is short
            chunks3 = [(i * CH, CH) for i in range(NCH - 1)]
            chunks3 += [(7 * CH, 1024), (7 * CH + 1024, 512), (7 * CH + 1536, 512)]
            for off, sz in chunks3:
                xm = p3m.tile([C_MID, CH], bf16, tag="xm")
                nc.sync.dma_start(out=xm[:, 0:sz], in_=xmid_d[:, off : off + sz])
                for half in range(2):
                    xt = p3x.tile([128, CH], bf16, tag=f"xt{half}")
                    nc.gpsimd.dma_start(
                        out=xt[:, 0:sz],
                        in_=x_d[half * 128 : (half + 1) * 128, off : off + sz],
                    )
                    ot = p3o.tile([128, CH], f32, tag=f"ot{half}")
                    for s in range(sz // 512):
                        po = psp.tile([128, 512], f32, tag="ps")
                        nc.tensor.matmul(
                            po[:],
                            w_outT_t[:, half * 128 : (half + 1) * 128],
                            xm[:, s * 512 : (s + 1) * 512],
                            start=True,
                            stop=True,
                        )
                        nc.vector.tensor_add(
                            ot[:, s * 512 : (s + 1) * 512],
                            po[:],
                            xt[:, s * 512 : (s + 1) * 512],
                        )
                    nc.scalar.dma_start(
                        out=out_d[half * 128 : (half + 1) * 128, off : off + sz],
                        in_=ot[:, 0:sz],
                    )

    nc.compile()
    _CACHE["nc"] = nc
    return nc


def _in_maps(x, w_in, w_out, bin_weights):
    import ml_dtypes

    bf = ml_dtypes.bfloat16
    wsel, frb, fib, fingb = _host_constants()
    x = np.ascontiguousarray(x, dtype=np.float32)
    shared = {
        "w_inT": np.ascontiguousarray(w_in.T.astype(bf)),
        "w_outT": np.ascontiguousarray(w_out.T.astype(bf)),
        "binT": np.ascontiguousarray(bin_weights.reshape(C_MID, AR).T.astype(bf)),
        "wsel": wsel,
        "fr": frb,
        "fi": fib,
        "fineg": fingb,
    }
    return [
        {"x": np.ascontiguousarray(x[b].reshape(C_IN, HW)), **shared}
        for b in range(B)
    ]


def _ensure_ntff_hook():
    """The agent image's antenv lacks axon_hooks; recreate it so
    run_bass_kernel_spmd(trace=True) can capture NTFF profiles."""
    import sys
    import types

    import antenv

    if hasattr(antenv, "axon_hooks"):
        return
    mod = types.ModuleType("antenv.axon_hooks")
    holder = [None]
    mod.set_axon_ntff_profile_hook = lambda h: holder.__setitem__(0, h)
    mod.get_axon_ntff_profile_hook = lambda: holder[0]
    sys.modules["antenv.axon_hooks"] = mod
    antenv.axon_hooks = mod
    try:
        from trn_agent_boot.trn_boot import _ntff_profile_via_ctypes

        mod.set_axon_ntff_profile_hook(
            _ntff_profile_via_ctypes("/opt/axon/libaxon_pjrt.so")
        )
    except Exception:
        pass


def run_on_device(x, w_in, w_out, bin_weights, trace=False):
    from concourse.bass_utils import run_bass_kernel_spmd

    if trace:
        _ensure_ntff_hook()
    nc = _build_nc()
    in_maps = _in_maps(
        np.asarray(x), np.asarray(w_in), np.asarray(w_out), np.asarray(bin_weights)
    )
    res = run_bass_kernel_spmd(nc, in_maps, list(range(B)), trace=trace)
    out = np.stack(
        [res.results[b]["out"].reshape(C_IN, H, W) for b in range(B)], axis=0
    )
    return out.astype(np.float32), res


def kernel(x, w_in, w_out, bin_weights):
    out, _ = run_on_device(x, w_in, w_out, bin_weights, trace=False)
    return out


# revision 26
# speedup vs baseline: 1.1918x; 1.0475x over previous
"""Trainium2 Bass kernel for nn_AngleFreqEnhance.

Reference computation (per batch element b):
    x_proj = w_in @ x_b                    # (16, 256) @ (256, 16384)
    Z      = fftshift(fft2(x_proj, ortho))
    enh    = (|Z|+eps) * gain * Z/|Z|      # == gain * Z up to ~1e-8 absolute
    x_enh  = ifft2(ifftshift(enh), ortho).real
    out_b  = x_b + w_out @ x_enh           # (256, 16) @ (16, 16384)

Device formulation:
  * eps term dropped (absolute error <= gain * 1e-8, far below the 2e-2 gate).
  * fftshift/ifftshift folded into the gain map (host-side ifftshift of the
    constant angle/radius selection maps).
  * 2-D FFT/IFFT done as matmuls with the symmetric 128x128 ortho DFT matrix
    F = Fr + i*Fi (bf16 operands, fp32 PSUM accumulate).  Each matmul stage
    contracts the partition axis and transposes the kept axis, so four stages
    (fwd row, fwd col, inv row, inv col) chain with no explicit transposes.
  * gain = bin_weights-flat @ Wsel, where Wsel[(a,r), pix] is the
    host-precomputed constant angle-weight * radius-indicator map
    (shape-derived constants only; bin_weights stays a device input).
  * All matmul operands are bf16 (full-rate TensorE + fast weight load); the
    residual x stream, gain map, PSUM accumulation, and output stay fp32, so
    bf16 rounding only touches the small enhancement term.

Sharding: pure data parallel - batch element b on core b (B=8, 8 cores).
No collectives. Host gathers per-core outputs.
"""

import math

import numpy as np

B, C_IN, C_MID, H, W = 8, 256, 16, 128, 128
HW = H * W
N_ANGLES = 8
N_RADII = 9
AR = N_ANGLES * N_RADII  # 72
CH = 2048                # pixel chunk for the streaming passes
NCH = HW // CH           # 8
NS = CH // 512           # 4 (PSUM-bank sized sub-chunks)

_CACHE = {}


def _build_masks():
    """Bit-faithful replica of the reference _build_masks on CPU jax.

    The reference's `(arctan2 + pi) % pi` lowers to an IEEE-remainder on
    XLA:CPU (range (-pi/2, pi/2]), which zeroes the angle weights over half
    the plane.  Running the identical jnp ops on the identical backend is the
    only safe way to reproduce the oracle.
    """
    import jax
    import jax.numpy as jnp

    EPS = 1e-8
    RADIUS_WIDTH = 8
    OVERLAP_RATIO = 1.5
    with jax.default_device(jax.devices("cpu")[0]):
        cy, cx = H // 2, W // 2
        y = jnp.arange(H, dtype=jnp.float32)[:, None] - cy
        x = jnp.arange(W, dtype=jnp.float32)[None, :] - cx
        r = jnp.sqrt(y * y + x * x)
        theta = (jnp.arctan2(y, x) + math.pi) % math.pi
        n_radii = int(max(cy, cx) // RADIUS_WIDTH) + 1
        radius_idx = jnp.clip(
            jnp.floor(r / RADIUS_WIDTH).astype(jnp.int32), 0, n_radii - 1
        )
        delta = math.pi / N_ANGLES
        half_width = OVERLAP_RATIO * delta / 2.0
        centers = (jnp.arange(N_ANGLES, dtype=jnp.float32) * delta + delta / 2.0)[
            :, None, None
        ]
        dist = jnp.abs(theta[None] - centers)
        w = jnp.clip(1.0 - dist / half_width, 0.0) * (dist < half_width)
        angle_weights = w / (w.sum(axis=0, keepdims=True) + EPS)
        return np.asarray(radius_idx), np.asarray(angle_weights)


def _host_constants():
    if "consts" in _CACHE:
        return _CACHE["consts"]
    import ml_dtypes

    bf = ml_dtypes.bfloat16
    radius_idx, aw = _build_masks()
    maps = aw[:, None] * (
        radius_idx[None, None] == np.arange(N_RADII)[None, :, None, None]
    ).astype(np.float32)
    wsel = (
        np.fft.ifftshift(maps, axes=(-2, -1))
        .reshape(AR, HW)
        .astype(np.float32)
    )
    k = np.arange(H)
    th = 2.0 * np.pi * np.outer(k, k) / H
    fr = (np.cos(th) / math.sqrt(H)).astype(np.float32)
    fi = (-np.sin(th) / math.sqrt(H)).astype(np.float32)
    consts = (
        np.ascontiguousarray(wsel.astype(bf)),
        np.ascontiguousarray(fr.astype(bf)),
        np.ascontiguousarray(fi.astype(bf)),
        np.ascontiguousarray((-fi).astype(bf)),
    )
    _CACHE["consts"] = consts
    return consts


def _build_nc():
    if "nc" in _CACHE:
        return _CACHE["nc"]
    import concourse.bass as bass
    import concourse.bacc as bacc
    import concourse.tile as tile
    from concourse import mybir

    f32 = mybir.dt.float32
    bf16 = mybir.dt.bfloat16
    PSUM = bass.MemorySpace.PSUM

    nc = bacc.Bacc(
        None,
        target_bir_lowering=False,
        debug=False,
        enable_asserts=False,
        num_devices=B,
    )

    x_d = nc.declare_dram_parameter("x", [C_IN, HW], f32, isOutput=False)
    w_inT_d = nc.declare_dram_parameter("w_inT", [C_IN, C_MID], bf16, isOutput=False)
    w_outT_d = nc.declare_dram_parameter("w_outT", [C_MID, C_IN], bf16, isOutput=False)
    binT_d = nc.declare_dram_parameter("binT", [AR, C_MID], bf16, isOutput=False)
    wsel_d = nc.declare_dram_parameter("wsel", [AR, HW], bf16, isOutput=False)
    fr_d = nc.declare_dram_parameter("fr", [H, H], bf16, isOutput=False)
    fi_d = nc.declare_dram_parameter("fi", [H, H], bf16, isOutput=False)
    fineg_d = nc.declare_dram_parameter("fineg", [H, H], bf16, isOutput=False)
    out_d = nc.declare_dram_parameter("out", [C_IN, HW], f32, isOutput=True)

    xproj_d = nc.dram_tensor("xproj_d", [C_MID, HW], bf16)
    xmid_d = nc.dram_tensor("xmid_d", [C_MID, HW], bf16)
    gain_d = nc.dram_tensor("gain_d", [C_MID, HW], bf16)

    xproj_r = xproj_d.ap().rearrange("m (h w) -> m h w", h=H)
    xmid_r = xmid_d.ap().rearrange("m (h w) -> m h w", h=H)
    gain_r = gain_d.ap().rearrange("m (h w) -> m h w", h=H)

    with tile.TileContext(nc) as tc:
        with (
            tc.tile_pool(name="const", bufs=1) as cpool,
            tc.tile_pool(name="gw", bufs=2) as gwp,
            tc.tile_pool(name="p1x", bufs=2) as p1x,
            tc.tile_pool(name="p1s", bufs=2) as p1s,
            tc.tile_pool(name="fftb", bufs=1) as fftb,
            tc.tile_pool(name="p3m", bufs=2) as p3m,
            tc.tile_pool(name="p3x", bufs=6) as p3x,
            tc.tile_pool(name="p3o", bufs=2) as p3o,
            tc.tile_pool(name="ps", bufs=8, space=PSUM) as psp,
        ):
            # ---- constants into SBUF
            w_inT_t = cpool.tile([128, 2 * C_MID], bf16)
            nc.sync.dma_start(out=w_inT_t[:, 0:C_MID], in_=w_inT_d[0:128, :])
            nc.sync.dma_start(out=w_inT_t[:, C_MID : 2 * C_MID], in_=w_inT_d[128:256, :])
            w_outT_t = cpool.tile([C_MID, C_IN], bf16)
            nc.sync.dma_start(out=w_outT_t[:], in_=w_outT_d[:])
            binT_t = cpool.tile([AR, C_MID], bf16)
            nc.sync.dma_start(out=binT_t[:], in_=binT_d[:])
            fr_t = cpool.tile([H, H], bf16)
            nc.sync.dma_start(out=fr_t[:], in_=fr_d[:])
            fi_t = cpool.tile([H, H], bf16)
            nc.sync.dma_start(out=fi_t[:], in_=fi_d[:])
            fng_t = cpool.tile([H, H], bf16)
            nc.sync.dma_start(out=fng_t[:], in_=fineg_d[:])
            # paired moving operands for batched FFT matmuls
            Fri = cpool.tile([H, 2 * H], bf16)   # [Fr | Fi]
            nc.sync.dma_start(out=Fri[:, 0:H], in_=fr_d[:])
            nc.sync.dma_start(out=Fri[:, H : 2 * H], in_=fi_d[:])
            Fnr = cpool.tile([H, 2 * H], bf16)   # [-Fi | Fr]
            nc.sync.dma_start(out=Fnr[:, 0:H], in_=fineg_d[:])
            nc.sync.dma_start(out=Fnr[:, H : 2 * H], in_=fr_d[:])
            Frn = cpool.tile([H, 2 * H], bf16)   # [Fr | -Fi]
            nc.sync.dma_start(out=Frn[:, 0:H], in_=fr_d[:])
            nc.sync.dma_start(out=Frn[:, H : 2 * H], in_=fineg_d[:])
            Fir = cpool.tile([H, 2 * H], bf16)   # [Fi | Fr]
            nc.sync.dma_start(out=Fir[:, 0:H], in_=fi_d[:])
            nc.sync.dma_start(out=Fir[:, H : 2 * H], in_=fr_d[:])

            # ---- phase 1: projection-down (streams all of x once)
            CH1 = 4096
            for c in range(HW // CH1):
                # SWDGE cast-DMA: fp32 HBM -> bf16 SBUF
                x0 = p1x.tile([128, CH1], bf16, tag="x0")
                x1 = p1x.tile([128, CH1], bf16, tag="x1")
                nc.gpsimd.dma_start(out=x0[:], in_=x_d[0:128, c * CH1 : (c + 1) * CH1])
                nc.gpsimd.dma_start(out=x1[:], in_=x_d[128:256, c * CH1 : (c + 1) * CH1])
                st = p1s.tile([C_MID, CH1], bf16, tag="p1st")
                for s in range(CH1 // 512):
                    pp = psp.tile([C_MID, 512], f32, tag="ps")
                    nc.tensor.matmul(
                        pp[:],
                        w_inT_t[:, 0:C_MID],
                        x0[:, s * 512 : (s + 1) * 512],
                        start=True,
                        stop=False,
                    )
                    nc.tensor.matmul(
                        pp[:],
                        w_inT_t[:, C_MID : 2 * C_MID],
                        x1[:, s * 512 : (s + 1) * 512],
                        start=False,
                        stop=True,
                    )
                    nc.vector.tensor_copy(st[:, s * 512 : (s + 1) * 512], pp[:])
                nc.scalar.dma_start(out=xproj_d[:, c * CH1 : (c + 1) * CH1], in_=st[:])

            # ---- gain table (rides the pass1 -> FFT boundary)
            for c in range(NCH):
                wt = gwp.tile([AR, CH], bf16)
                nc.sync.dma_start(out=wt[:], in_=wsel_d[:, c * CH : (c + 1) * CH])
                gs = p1s.tile([C_MID, CH], bf16, tag="gs")
                for s in range(NS):
                    pg = psp.tile([C_MID, 512], f32, tag="ps")
                    nc.tensor.matmul(
                        pg[:],
                        binT_t[:],
                        wt[:, s * 512 : (s + 1) * 512],
                        start=True,
                        stop=True,
                    )
                    nc.scalar.copy(gs[:, s * 512 : (s + 1) * 512], pg[:])
                nc.scalar.dma_start(out=gain_d[:, c * CH : (c + 1) * CH], in_=gs[:])

            # ---- phase 2: FFT -> gain -> IFFT, stage-major, 2 channels per bank
            # bulk one-shot reshape loads: [m, h*w] -> [h, m*128+w]
            Xall = fftb.tile([H, C_MID * H], bf16)
            gall = fftb.tile([H, C_MID * H], bf16)
            nc.sync.dma_start(
                out=Xall[:], in_=xproj_d.ap().rearrange("m (h w) -> h m w", h=H)
            )
            nc.sync.dma_start(
                out=gall[:], in_=gain_d.ap().rearrange("m (h w) -> h m w", h=H)
            )

            sAall = fftb.tile([H, C_MID * 2 * H], bf16)
            sEall = fftb.tile([H, C_MID * 2 * H], bf16)
            sUall = fftb.tile([H, C_MID * 2 * H], bf16)
            sXall = fftb.tile([H, C_MID * H], bf16)
            NP = C_MID // 2  # channel pairs

            for q in range(NP):
                pA = psp.tile([H, 4 * H], f32, tag="ps")
                for j in range(2):
                    Xm = Xall[:, (2 * q + j) * H : (2 * q + j + 1) * H]
                    nc.tensor.matmul(
                        pA[:, 2 * j * H : (2 * j + 2) * H], Xm, Fri[:],
                        start=True, stop=True,
                    )
                engA = nc.scalar.copy if q % 4 == 3 else nc.vector.tensor_copy
                engA(sAall[:, q * 4 * H : (q + 1) * 4 * H], pA[:])

            for q in range(NP):
                pB = psp.tile([H, 4 * H], f32, tag="ps")
                for j in range(2):
                    m = 2 * q + j
                    sAr = sAall[:, m * 2 * H : m * 2 * H + H]
                    sAi = sAall[:, m * 2 * H + H : (m + 1) * 2 * H]
                    bri = pB[:, 2 * j * H : (2 * j + 2) * H]
                    nc.tensor.matmul(bri, sAr, Fri[:], start=True, stop=False)
                    nc.tensor.matmul(bri, sAi, Fnr[:], start=False, stop=True)
                gq = (
                    gall[:, 2 * q * H : (2 * q + 2) * H]
                    .rearrange("p (m w) -> p m w", m=2)
                    .unsqueeze(2)
                    .broadcast_to((H, 2, 2, H))
                )
                nc.vector.tensor_mul(
                    sEall[:, q * 4 * H : (q + 1) * 4 * H].rearrange(
                        "p (m r w) -> p m r w", m=2, r=2
                    ),
                    pB[:].rearrange("p (m r w) -> p m r w", m=2, r=2),
                    gq,
                )

            for q in range(NP):
                pC = psp.tile([H, 4 * H], f32, tag="ps")
                for j in range(2):
                    m = 2 * q + j
                    sEr = sEall[:, m * 2 * H : m * 2 * H + H]
                    sEi = sEall[:, m * 2 * H + H : (m + 1) * 2 * H]
                    cri = pC[:, 2 * j * H : (2 * j + 2) * H]
                    nc.tensor.matmul(cri, sEr, Frn[:], start=True, stop=False)
                    nc.tensor.matmul(cri, sEi, Fir[:], start=False, stop=True)
                engC = nc.scalar.copy if q % 4 == 3 else nc.vector.tensor_copy
                engC(sUall[:, q * 4 * H : (q + 1) * 4 * H], pC[:])

            for q in range(NP):
                pD = psp.tile([H, 2 * H], f32, tag="ps")
                for j in range(2):
                    m = 2 * q + j
                    sUr = sUall[:, m * 2 * H : m * 2 * H + H]
                    sUi = sUall[:, m * 2 * H + H : (m + 1) * 2 * H]
                    xe = pD[:, j * H : (j + 1) * H]
                    nc.tensor.matmul(xe, sUr, fr_t[:], start=True, stop=False)
                    nc.tensor.matmul(xe, sUi, fi_t[:], start=False, stop=True)
                engD = nc.scalar.copy if q % 4 == 3 else nc.vector.tensor_copy
                engD(sXall[:, q * 2 * H : (q + 1) * 2 * H], pD[:])
                nc.scalar.dma_start(
                    out=xmid_d.ap().rearrange("m (h w) -> h m w", h=H)[
                        :, 2 * q : 2 * q + 2, :
                    ],
                    in_=sXall[:, q * 2 * H : (q + 1) * 2 * H],
                )

            # ---- phase 3: projection-up + residual (streams x again + out)
            # bf16 residual stream via the otherwise-idle SWDGE ring; tapered
            # final chunks so the write drain is short
            chunks3 = [(i * CH, CH) for i in range(NCH - 1)]
            chunks3 += [(7 * CH, 1024), (7 * CH + 1024, 512), (7 * CH + 1536, 512)]
            for off, sz in chunks3:
                xm = p3m.tile([C_MID, CH], bf16, tag="xm")
                nc.sync.dma_start(out=xm[:, 0:sz], in_=xmid_d[:, off : off + sz])
                for half in range(2):
                    xt = p3x.tile([128, CH], bf16, tag=f"xt{half}")
                    nc.gpsimd.dma_start(
                        out=xt[:, 0:sz],
                        in_=x_d[half * 128 : (half + 1) * 128, off : off + sz],
                    )
                    ot = p3o.tile([128, CH], f32, tag=f"ot{half}")
                    for s in range(sz // 512):
                        po = psp.tile([128, 512], f32, tag="ps")
                        nc.tensor.matmul(
                            po[:],
                            w_outT_t[:, half * 128 : (half + 1) * 128],
                            xm[:, s * 512 : (s + 1) * 512],
                            start=True,
                            stop=True,
                        )
                        nc.vector.tensor_add(
                            ot[:, s * 512 : (s + 1) * 512],
                            po[:],
                            xt[:, s * 512 : (s + 1) * 512],
                        )
                    nc.scalar.dma_start(
                        out=out_d[half * 128 : (half + 1) * 128, off : off + sz],
                        in_=ot[:, 0:sz],
                    )

    nc.compile()
    _CACHE["nc"] = nc
    return nc


def _in_maps(x, w_in, w_out, bin_weights):
    import ml_dtypes

    bf = ml_dtypes.bfloat16
    wsel, frb, fib, fingb = _host_constants()
    x = np.ascontiguousarray(x, dtype=np.float32)
    shared = {
        "w_inT": np.ascontiguousarray(w_in.T.astype(bf)),
        "w_outT": np.ascontiguousarray(w_out.T.astype(bf)),
        "binT": np.ascontiguousarray(bin_weights.reshape(C_MID, AR).T.astype(bf)),
        "wsel": wsel,
        "fr": frb,
        "fi": fib,
        "fineg": fingb,
    }
    return [
        {"x": np.ascontiguousarray(x[b].reshape(C_IN, HW)), **shared}
        for b in range(B)
    ]


def _ensure_ntff_hook():
    """The agent image's antenv lacks axon_hooks; recreate it so
    run_bass_kernel_spmd(trace=True) can capture NTFF profiles."""
    import sys
    import types

    import antenv

    if hasattr(antenv, "axon_hooks"):
        return
    mod = types.ModuleType("antenv.axon_hooks")
    holder = [None]
    mod.set_axon_ntff_profile_hook = lambda h: holder.__setitem__(0, h)
    mod.get_axon_ntff_profile_hook = lambda: holder[0]
    sys.modules["antenv.axon_hooks"] = mod
    antenv.axon_hooks = mod
    try:
        from trn_agent_boot.trn_boot import _ntff_profile_via_ctypes

        mod.set_axon_ntff_profile_hook(
            _ntff_profile_via_ctypes("/opt/axon/libaxon_pjrt.so")
        )
    except Exception:
        pass


def run_on_device(x, w_in, w_out, bin_weights, trace=False):
    from concourse.bass_utils import run_bass_kernel_spmd

    if trace:
        _ensure_ntff_hook()
    nc = _build_nc()
    in_maps = _in_maps(
        np.asarray(x), np.asarray(w_in), np.asarray(w_out), np.asarray(bin_weights)
    )
    res = run_bass_kernel_spmd(nc, in_maps, list(range(B)), trace=trace)
    out = np.stack(
        [res.results[b]["out"].reshape(C_IN, H, W) for b in range(B)], axis=0
    )
    return out.astype(np.float32), res


def kernel(x, w_in, w_out, bin_weights):
    out, _ = run_on_device(x, w_in, w_out, bin_weights, trace=False)
    return out
